# revision 21
# baseline (speedup 1.0000x reference)
"""AlphaRotatedIoULoss on 8 TRN2 NeuronCores (raw Bass SPMD kernel).

Sort-free replication of the reference's rotated-IoU loss:
  - intersection area via directed-segment shoelace: clip each box's 4 edges
    against the other box (branch-free Liang-Barsky in that box's local
    frame), then sum span*cross(k,d) over the 8 directed boundary segments
    (all expressed in box2's frame).
  - the reference's shoelace drops the closing edge (last->first angle-sorted
    vertex) because invalid candidate slots are zeroed; the missing term is
    the cross of the unique boundary segment crossing the global -x ray from
    the vertex centroid.  Replicated branch-free via a global-y sign test.
Data-parallel over boxes: each core takes 1/8th, emits per-partition partial
sums of weight*iou^3; host combines:  loss = (sum(w) - total) / n.
"""
import numpy as np

P = 128          # partitions
T = 496          # boxes per partition per core
F = 124          # chunk width (free dim)
NCHUNK = T // F  # 4
NCORE = P * T    # 63488 boxes per core
NSHARD = 62500   # real boxes per core
NGLOB = 500000
NCORES = 8
ALPHA_EPS = 1e-6
TINY = 1e-12

_cache = {}


def _build_graph():
    import concourse.bass as bass
    import concourse.mybir as mybir
    from contextlib import ExitStack

    dt = mybir.dt.float32
    AF = mybir.ActivationFunctionType
    OP = mybir.AluOpType
    AX = mybir.AxisListType
    HALF_PI = float(np.pi / 2)

    nc = bass.Bass(detect_race_conditions=False)

    def reg_const(value):
        t = nc.alloc_sbuf_tensor(f"const-f32-{value}", [128, 1], dt)
        nc.gpsimd.memset(t.ap(), value)
        nc.const_aps.aps[(dt, float(value))] = t.ap()

    reg_const(HALF_PI)
    nc.all_engine_barrier()

    pred = nc.declare_dram_parameter("pred", [NCORE, 5], dt, isOutput=False)
    targ = nc.declare_dram_parameter("target", [NCORE, 5], dt, isOutput=False)
    wgt = nc.declare_dram_parameter("weight", [NCORE], dt, isOutput=False)
    out = nc.declare_dram_parameter("out", [P, NCHUNK], dt, isOutput=True)

    predv = pred.rearrange("(p t) f -> p t f", p=P)
    targv = targ.rearrange("(p t) f -> p t f", p=P)
    wv = wgt.rearrange("(p t) -> p t", p=P)

    V = nc.vector
    A = nc.scalar

    with ExitStack() as ctx:
        _n = [0]

        def alloc(shape):
            _n[0] += 1
            return ctx.enter_context(
                nc.sbuf_tensor(f"tile{_n[0]}", shape, dt))

        # double-buffered inputs
        pt2 = [alloc([P, F, 5]) for _ in range(2)]
        tg2 = [alloc([P, F, 5]) for _ in range(2)]
        wt2 = [alloc([P, F]) for _ in range(2)]
        # ACT-owned trig outputs (per chunk, single buffer)
        sdr_t, cd_t, s1_t, c1_t, s2_t, c2_t, sg_t = (alloc([P, F]) for _ in range(7))
        # F-sized DVE scratch
        FNAMES = ("delta a2y a2g a2w asd sd px py wcf wsf hcf hsf vcf vsf gcf gsf "
                  "awc aws ahc ahs bwc bws bhc bhs W1 H1 W2 H2 u1 u2 mu v1 v2 mv "
                  "z1 z2 nu z3 z4 nv e1 e2 f1 f2 p1_ p2_ q1_ q2_ m_ sx sy i2m cx cy "
                  "wsg hcg awsg ahcg f1g f2g vsg gcg bvsg bgcg h1g h2g cgy t_a t_b "
                  "S_ CR_ inter ar1 ar2 un ru iou io2 io3 junk").split()
        FT = {n: alloc([P, F]) for n in FNAMES}
        # 4F clip scratch (shared between part A and part B)
        C4 = {n: alloc([P, 4 * F]) for n in
              "KXP KXM KYP KYM RX RY TXP TXQ TYP TYQ TEX TXX TEY TYY GX4 GY4".split()}
        # 8F segment tiles
        E8 = {n: alloc([P, 8 * F]) for n in
              ("K8X K8Y D8X D8Y TLO8 THI8 SPAN8 CRK TMP8 SV8 VAL8 TS8 BX8 BY8 "
               "KGY DGY TAU_A TAU_B HIT8 CHI").split()}
        acc4 = alloc([P, NCHUNK])

        with (
            nc.semaphore("dma_sem") as dma_sem,
            nc.semaphore("v_sem") as v_sem,
            nc.semaphore("a_sem") as a_sem,
            nc.semaphore("r_sem") as r_sem,
            nc.semaphore("done_sem") as done_sem,
            nc.Block() as block,
        ):
            @block.sync
            def _(sync):
                for ch in range(NCHUNK):
                    if ch >= 2:
                        # wait until DVE is done reading this buffer pair
                        sync.wait_ge(r_sem, ch - 1)
                    b = ch % 2
                    sync.dma_start(
                        out=pt2[b][:], in_=predv[:, ch * F:(ch + 1) * F, :]
                    ).then_inc(dma_sem, 16)
                    sync.dma_start(
                        out=tg2[b][:], in_=targv[:, ch * F:(ch + 1) * F, :]
                    ).then_inc(dma_sem, 16)
                    sync.dma_start(
                        out=wt2[b][:], in_=wv[:, ch * F:(ch + 1) * F]
                    ).then_inc(dma_sem, 16)
                sync.wait_ge(done_sem, 1)
                sync.dma_start(out=out[:], in_=acc4[:]).then_inc(dma_sem, 16)

            @block.scalar
            def _(scalar):
                for ch in range(NCHUNK):
                    b = ch % 2
                    a1 = pt2[b][:, :, 4]
                    a2 = tg2[b][:, :, 4]
                    scalar.wait_ge(v_sem, ch + 1)
                    A.activation(sdr_t[:], FT["delta"][:], AF.Sin)
                    A.activation(cd_t[:], FT["delta"][:], AF.Sin, bias=HALF_PI)
                    A.activation(s1_t[:], a1, AF.Sin)
                    A.activation(c1_t[:], a1, AF.Sin, bias=HALF_PI)
                    A.activation(s2_t[:], a2, AF.Sin)
                    A.activation(c2_t[:], FT["a2w"][:], AF.Sin)
                    A.drain().then_inc(a_sem, 1)

            @block.vector
            def _(vector):
                t = lambda n: FT[n][:]
                c = lambda n: C4[n][:]
                e = lambda n: E8[n][:]

                def sl(t8, i):
                    return E8[t8][:, i * F:(i + 1) * F]

                def sl4(t4, i):
                    return C4[t4][:, i * F:(i + 1) * F]

                # constant zero slices of D8X/D8Y (box2's own AA edge dirs)
                V.memset(sl("D8X", 5), 0.0)
                V.memset(sl("D8X", 7), 0.0)
                V.memset(sl("D8Y", 4), 0.0)
                V.memset(sl("D8Y", 6), 0.0)

                for ch in range(NCHUNK):
                    b = ch % 2
                    pt, tg, wt = pt2[b], tg2[b], wt2[b]
                    x1, y1, w1, h1, a1 = (pt[:, :, i] for i in range(5))
                    x2, y2, w2, h2, a2 = (tg[:, :, i] for i in range(5))

                    vector.wait_ge(dma_sem, 48 * (ch + 1))
                    # angles for ACT: delta and wrapped a2+pi/2
                    V.tensor_tensor(t("delta"), a1, a2, OP.subtract)
                    V.tensor_scalar(t("a2y"), a2, HALF_PI, None, OP.add)
                    V.tensor_scalar(t("a2g"), t("a2y"), float(np.pi), None, OP.is_gt)
                    V.scalar_tensor_tensor(
                        t("a2w"), t("a2g"), float(-2 * np.pi), t("a2y"),
                        OP.mult, OP.add)
                    V.drain().then_inc(v_sem, 1)

                    # independent of trig: px/py, bounds, areas
                    V.tensor_tensor(t("px"), x2, x1, OP.subtract)
                    V.tensor_tensor(t("py"), y2, y1, OP.subtract)
                    V.tensor_scalar(t("W1"), w1, 0.5, None, OP.mult)
                    V.tensor_scalar(t("H1"), h1, 0.5, None, OP.mult)
                    V.tensor_scalar(t("W2"), w2, 0.5, None, OP.mult)
                    V.tensor_scalar(t("H2"), h2, 0.5, None, OP.mult)
                    V.tensor_tensor(t("ar1"), w1, h1, OP.mult)
                    V.tensor_tensor(t("ar2"), w2, h2, OP.mult)
                    # box2 own corners in K8X/K8Y slices 4..7
                    V.tensor_copy(sl("K8X", 4), t("W2"))
                    V.tensor_scalar(sl("K8X", 5), w2, -0.5, None, OP.mult)
                    V.tensor_scalar(sl("K8X", 6), w2, -0.5, None, OP.mult)
                    V.tensor_copy(sl("K8X", 7), t("W2"))
                    V.tensor_copy(sl("K8Y", 4), t("H2"))
                    V.tensor_copy(sl("K8Y", 5), t("H2"))
                    V.tensor_scalar(sl("K8Y", 6), h2, -0.5, None, OP.mult)
                    V.tensor_scalar(sl("K8Y", 7), h2, -0.5, None, OP.mult)
                    # box2 own edge dirs in D8X/D8Y slices 4..7
                    V.tensor_scalar(sl("D8X", 4), w2, -1.0, None, OP.mult)
                    V.tensor_copy(sl("D8X", 6), w2)
                    V.tensor_scalar(sl("D8Y", 5), h2, -1.0, None, OP.mult)
                    V.tensor_copy(sl("D8Y", 7), h2)

                    # trig-dependent section
                    vector.wait_ge(a_sem, ch + 1)
                    V.tensor_scalar(sg_t[:], sdr_t[:], 0.0, None, OP.is_ge)
                    V.tensor_scalar(sg_t[:], sg_t[:], 2.0, -1.0,
                                    OP.mult, OP.add)
                    V.tensor_tensor(t("asd"), sg_t[:], sdr_t[:], OP.mult)
                    V.tensor_scalar(t("asd"), t("asd"), TINY, None, OP.max)
                    V.tensor_tensor(t("sd"), sg_t[:], t("asd"), OP.mult)
                    cdA, sdA = cd_t[:], t("sd")
                    s1A, c1A, s2A, c2A = s1_t[:], c1_t[:], s2_t[:], c2_t[:]

                    V.tensor_tensor(t("wcf"), w1, cdA, OP.mult)
                    V.tensor_tensor(t("wsf"), w1, sdA, OP.mult)
                    V.tensor_tensor(t("hcf"), h1, cdA, OP.mult)
                    V.tensor_tensor(t("hsf"), h1, sdA, OP.mult)
                    V.tensor_tensor(t("vcf"), w2, cdA, OP.mult)
                    V.tensor_tensor(t("vsf"), w2, sdA, OP.mult)
                    V.tensor_tensor(t("gcf"), h2, cdA, OP.mult)
                    V.tensor_tensor(t("gsf"), h2, sdA, OP.mult)
                    for full, half in (("wcf", "awc"), ("wsf", "aws"),
                                       ("hcf", "ahc"), ("hsf", "ahs"),
                                       ("vcf", "bwc"), ("vsf", "bws"),
                                       ("gcf", "bhc"), ("gsf", "bhs")):
                        V.tensor_scalar(t(half), t(full), 0.5, None, OP.mult)

                    # centers
                    V.tensor_tensor(t("u1"), t("px"), c2A, OP.mult)
                    V.tensor_tensor(t("u2"), t("py"), s2A, OP.mult)
                    V.scalar_tensor_tensor(t("mu"), t("u1"), -1.0, t("u2"),
                                           OP.mult, OP.subtract)
                    V.tensor_tensor(t("v1"), t("px"), s2A, OP.mult)
                    V.tensor_tensor(t("v2"), t("py"), c2A, OP.mult)
                    V.tensor_tensor(t("mv"), t("v1"), t("v2"), OP.subtract)
                    V.tensor_tensor(t("z1"), t("px"), c1A, OP.mult)
                    V.tensor_tensor(t("z2"), t("py"), s1A, OP.mult)
                    V.tensor_tensor(t("nu"), t("z1"), t("z2"), OP.add)
                    V.tensor_tensor(t("z3"), t("px"), s1A, OP.mult)
                    V.tensor_tensor(t("z4"), t("py"), c1A, OP.mult)
                    V.tensor_tensor(t("nv"), t("z4"), t("z3"), OP.subtract)

                    # box1 corners in box2 frame -> K8X/K8Y slices 0..3
                    V.tensor_tensor(t("e1"), t("awc"), t("ahs"), OP.subtract)
                    V.tensor_tensor(t("e2"), t("awc"), t("ahs"), OP.add)
                    V.tensor_tensor(sl("K8X", 0), t("mu"), t("e1"), OP.add)
                    V.tensor_tensor(sl("K8X", 1), t("mu"), t("e2"), OP.subtract)
                    V.tensor_tensor(sl("K8X", 2), t("mu"), t("e1"), OP.subtract)
                    V.tensor_tensor(sl("K8X", 3), t("mu"), t("e2"), OP.add)
                    V.tensor_tensor(t("f1"), t("aws"), t("ahc"), OP.add)
                    V.tensor_tensor(t("f2"), t("aws"), t("ahc"), OP.subtract)
                    V.tensor_tensor(sl("K8Y", 0), t("mv"), t("f1"), OP.add)
                    V.tensor_tensor(sl("K8Y", 1), t("mv"), t("f2"), OP.subtract)
                    V.tensor_tensor(sl("K8Y", 2), t("mv"), t("f1"), OP.subtract)
                    V.tensor_tensor(sl("K8Y", 3), t("mv"), t("f2"), OP.add)
                    # box1 edge dirs in box2 frame -> D8X/D8Y slices 0..3
                    V.tensor_scalar(sl("D8X", 0), t("wcf"), -1.0, None, OP.mult)
                    V.tensor_copy(sl("D8X", 1), t("hsf"))
                    V.tensor_copy(sl("D8X", 2), t("wcf"))
                    V.tensor_scalar(sl("D8X", 3), t("hsf"), -1.0, None, OP.mult)
                    V.tensor_scalar(sl("D8Y", 0), t("wsf"), -1.0, None, OP.mult)
                    V.tensor_scalar(sl("D8Y", 1), t("hcf"), -1.0, None, OP.mult)
                    V.tensor_copy(sl("D8Y", 2), t("wsf"))
                    V.tensor_copy(sl("D8Y", 3), t("hcf"))

                    def emit_clip(KXv, KYv, Wb, Hb, r0, r1, r0y, r1y,
                                  rx_spec, ry_spec, lo_out, hi_out, span_out):
                        for i in range(4):
                            kx_i = KXv[:, i * F:(i + 1) * F]
                            ky_i = KYv[:, i * F:(i + 1) * F]
                            V.tensor_tensor(sl4("KXP", i), kx_i, Wb, OP.add)
                            V.tensor_tensor(sl4("KXM", i), kx_i, Wb, OP.subtract)
                            V.tensor_tensor(sl4("KYP", i), ky_i, Hb, OP.add)
                            V.tensor_tensor(sl4("KYM", i), ky_i, Hb, OP.subtract)
                        for Rt, spec, ra, rb in (("RX", rx_spec, r0, r1),
                                                 ("RY", ry_spec, r0y, r1y)):
                            (p0s, n0s), (p1s, n1s) = spec
                            V.reciprocal(sl4(Rt, p0s), ra)
                            V.reciprocal(sl4(Rt, p1s), rb)
                            V.tensor_scalar(sl4(Rt, n0s), sl4(Rt, p0s), -1.0,
                                            None, OP.mult)
                            V.tensor_scalar(sl4(Rt, n1s), sl4(Rt, p1s), -1.0,
                                            None, OP.mult)
                        V.tensor_tensor(c("TXP"), c("KXP"), c("RX"), OP.mult)
                        V.tensor_tensor(c("TXQ"), c("KXM"), c("RX"), OP.mult)
                        V.tensor_tensor(c("TYP"), c("KYP"), c("RY"), OP.mult)
                        V.tensor_tensor(c("TYQ"), c("KYM"), c("RY"), OP.mult)
                        V.tensor_tensor(c("TEX"), c("TXP"), c("TXQ"), OP.min)
                        V.tensor_tensor(c("TXX"), c("TXP"), c("TXQ"), OP.max)
                        V.tensor_tensor(c("TEY"), c("TYP"), c("TYQ"), OP.min)
                        V.tensor_tensor(c("TYY"), c("TYP"), c("TYQ"), OP.max)
                        V.tensor_tensor(lo_out, c("TEX"), c("TEY"), OP.max)
                        V.tensor_scalar(lo_out, lo_out, 0.0, None, OP.max)
                        V.tensor_tensor(hi_out, c("TXX"), c("TYY"), OP.min)
                        V.tensor_scalar(hi_out, hi_out, 1.0, None, OP.min)
                        V.scalar_tensor_tensor(span_out, lo_out, -1.0, hi_out,
                                               OP.mult, OP.add)
                        V.tensor_scalar(span_out, span_out, 0.0, None, OP.max)

                    # part A: box1 edges vs AA box2
                    emit_clip(E8["K8X"][:, 0:4 * F], E8["K8Y"][:, 0:4 * F],
                              t("W2"), t("H2"),
                              t("wcf"), t("hsf"), t("wsf"), t("hcf"),
                              ((0, 2), (3, 1)), ((0, 2), (1, 3)),
                              E8["TLO8"][:, 0:4 * F], E8["THI8"][:, 0:4 * F],
                              E8["SPAN8"][:, 0:4 * F])

                    # part B corners (box2 in box1 frame)
                    V.tensor_tensor(t("p1_"), t("bwc"), t("bhs"), OP.add)
                    V.tensor_tensor(t("p2_"), t("bwc"), t("bhs"), OP.subtract)
                    V.tensor_tensor(sl4("GX4", 0), t("nu"), t("p1_"), OP.add)
                    V.tensor_tensor(sl4("GX4", 1), t("nu"), t("p2_"), OP.subtract)
                    V.tensor_tensor(sl4("GX4", 2), t("nu"), t("p1_"), OP.subtract)
                    V.tensor_tensor(sl4("GX4", 3), t("nu"), t("p2_"), OP.add)
                    V.tensor_tensor(t("q1_"), t("bhc"), t("bws"), OP.subtract)
                    V.tensor_tensor(t("q2_"), t("bhc"), t("bws"), OP.add)
                    V.tensor_tensor(sl4("GY4", 0), t("nv"), t("q1_"), OP.add)
                    V.tensor_tensor(sl4("GY4", 1), t("nv"), t("q2_"), OP.add)
                    V.tensor_tensor(sl4("GY4", 2), t("nv"), t("q1_"), OP.subtract)
                    V.tensor_tensor(sl4("GY4", 3), t("nv"), t("q2_"), OP.subtract)

                    # part B: box2 edges vs AA box1 (t-values frame-invariant)
                    emit_clip(c("GX4"), c("GY4"), t("W1"), t("H1"),
                              t("vcf"), t("gsf"), t("vsf"), t("gcf"),
                              ((0, 2), (1, 3)), ((2, 0), (1, 3)),
                              E8["TLO8"][:, 4 * F:8 * F],
                              E8["THI8"][:, 4 * F:8 * F],
                              E8["SPAN8"][:, 4 * F:8 * F])

                    # area terms
                    V.tensor_tensor(e("CRK"), e("K8X"), e("D8Y"), OP.mult)
                    V.tensor_tensor(e("TMP8"), e("K8Y"), e("D8X"), OP.mult)
                    V.tensor_tensor(e("CRK"), e("CRK"), e("TMP8"), OP.subtract)
                    V.tensor_tensor(e("SV8"), e("SPAN8"), e("CRK"), OP.mult)

                    # vertex centroid
                    V.tensor_scalar(e("VAL8"), e("SPAN8"), 0.0, None, OP.is_gt)
                    V.tensor_tensor(e("TS8"), e("TLO8"), e("THI8"), OP.add)
                    V.tensor_tensor(e("BX8"), e("TS8"), e("D8X"), OP.mult)
                    V.scalar_tensor_tensor(e("BX8"), e("K8X"), 2.0, e("BX8"),
                                           OP.mult, OP.add)
                    V.tensor_tensor(e("BX8"), e("BX8"), e("VAL8"), OP.mult)
                    V.tensor_tensor(e("BY8"), e("TS8"), e("D8Y"), OP.mult)
                    V.scalar_tensor_tensor(e("BY8"), e("K8Y"), 2.0, e("BY8"),
                                           OP.mult, OP.add)
                    V.tensor_tensor(e("BY8"), e("BY8"), e("VAL8"), OP.mult)

                    def seg_reduce(dst, src8):
                        v = E8[src8][:].rearrange("p (s f) -> p f s", s=8)
                        V.tensor_reduce(dst, v, AX.X, OP.add)

                    seg_reduce(t("m_"), "VAL8")
                    seg_reduce(t("sx"), "BX8")
                    seg_reduce(t("sy"), "BY8")
                    V.tensor_scalar(t("i2m"), t("m_"), 2.0, 1.0, OP.mult, OP.max)
                    V.reciprocal(t("i2m"), t("i2m"))
                    V.tensor_tensor(t("cx"), t("sx"), t("i2m"), OP.mult)
                    V.tensor_tensor(t("cy"), t("sy"), t("i2m"), OP.mult)

                    # global-y of segment starts/dirs (rel box2 center)
                    V.tensor_tensor(t("wsg"), w1, s1A, OP.mult)
                    V.tensor_tensor(t("hcg"), h1, c1A, OP.mult)
                    V.tensor_scalar(t("awsg"), t("wsg"), 0.5, None, OP.mult)
                    V.tensor_scalar(t("ahcg"), t("hcg"), 0.5, None, OP.mult)
                    V.tensor_tensor(t("f1g"), t("awsg"), t("ahcg"), OP.add)
                    V.tensor_tensor(t("f2g"), t("ahcg"), t("awsg"), OP.subtract)
                    V.tensor_tensor(t("vsg"), w2, s2A, OP.mult)
                    V.tensor_tensor(t("gcg"), h2, c2A, OP.mult)
                    V.tensor_scalar(t("bvsg"), t("vsg"), 0.5, None, OP.mult)
                    V.tensor_scalar(t("bgcg"), t("gcg"), 0.5, None, OP.mult)
                    V.tensor_tensor(t("h1g"), t("bvsg"), t("bgcg"), OP.add)
                    V.tensor_tensor(t("h2g"), t("bgcg"), t("bvsg"), OP.subtract)

                    V.tensor_tensor(sl("KGY", 0), t("f1g"), t("py"), OP.subtract)
                    V.tensor_tensor(sl("KGY", 1), t("f2g"), t("py"), OP.subtract)
                    V.scalar_tensor_tensor(sl("KGY", 2), t("f1g"), -1.0, t("py"),
                                           OP.mult, OP.subtract)
                    V.scalar_tensor_tensor(sl("KGY", 3), t("f2g"), -1.0, t("py"),
                                           OP.mult, OP.subtract)
                    V.tensor_copy(sl("KGY", 4), t("h1g"))
                    V.tensor_copy(sl("KGY", 5), t("h2g"))
                    V.tensor_scalar(sl("KGY", 6), t("h1g"), -1.0, None, OP.mult)
                    V.tensor_scalar(sl("KGY", 7), t("h2g"), -1.0, None, OP.mult)
                    V.tensor_scalar(sl("DGY", 0), t("wsg"), -1.0, None, OP.mult)
                    V.tensor_scalar(sl("DGY", 1), t("hcg"), -1.0, None, OP.mult)
                    V.tensor_copy(sl("DGY", 2), t("wsg"))
                    V.tensor_copy(sl("DGY", 3), t("hcg"))
                    V.tensor_scalar(sl("DGY", 4), t("vsg"), -1.0, None, OP.mult)
                    V.tensor_scalar(sl("DGY", 5), t("gcg"), -1.0, None, OP.mult)
                    V.tensor_copy(sl("DGY", 6), t("vsg"))
                    V.tensor_copy(sl("DGY", 7), t("gcg"))

                    # centroid global-y; tau tests; correction
                    V.tensor_tensor(t("t_a"), s2A, t("cx"), OP.mult)
                    V.tensor_tensor(t("t_b"), c2A, t("cy"), OP.mult)
                    V.tensor_tensor(t("cgy"), t("t_a"), t("t_b"), OP.add)
                    V.tensor_tensor(e("TAU_A"), e("TLO8"), e("DGY"), OP.mult)
                    V.tensor_tensor(e("TAU_B"), e("SPAN8"), e("DGY"), OP.mult)
                    for i in range(8):
                        # TMP8 slice = KGY - cgy
                        V.tensor_tensor(sl("TMP8", i), sl("KGY", i), t("cgy"),
                                        OP.subtract)
                    V.tensor_tensor(e("TAU_A"), e("TAU_A"), e("TMP8"), OP.add)
                    V.tensor_tensor(e("TAU_B"), e("TAU_B"), e("TAU_A"), OP.add)
                    V.tensor_scalar(e("HIT8"), e("TAU_A"), 0.0, None, OP.is_ge)
                    V.tensor_scalar(e("TMP8"), e("TAU_B"), 0.0, None, OP.is_lt)
                    V.tensor_tensor(e("HIT8"), e("HIT8"), e("TMP8"), OP.mult)
                    for i in range(8):
                        V.tensor_tensor(sl("CHI", i), t("cx"), sl("D8Y", i),
                                        OP.mult)
                        V.tensor_tensor(sl("TMP8", i), t("cy"), sl("D8X", i),
                                        OP.mult)
                    V.tensor_tensor(e("CHI"), e("CHI"), e("TMP8"), OP.subtract)
                    V.tensor_tensor(e("CHI"), e("SPAN8"), e("CHI"), OP.mult)
                    V.tensor_tensor(e("CHI"), e("SV8"), e("CHI"), OP.subtract)
                    V.tensor_tensor(e("CHI"), e("HIT8"), e("CHI"), OP.mult)

                    seg_reduce(t("S_"), "SV8")
                    seg_reduce(t("CR_"), "CHI")
                    V.tensor_tensor(t("S_"), t("S_"), t("CR_"), OP.subtract)

                    # iou / loss
                    V.tensor_scalar(t("inter"), t("S_"), 0.5, 0.0,
                                    OP.mult, OP.max)
                    V.tensor_tensor(t("un"), t("ar1"), t("ar2"), OP.add)
                    V.tensor_tensor(t("un"), t("un"), t("inter"), OP.subtract)
                    V.tensor_scalar(t("un"), t("un"), ALPHA_EPS, None, OP.max)
                    V.reciprocal(t("ru"), t("un"))
                    V.tensor_tensor(t("iou"), t("inter"), t("ru"), OP.mult)
                    V.tensor_scalar(t("iou"), t("iou"), ALPHA_EPS, None, OP.max)
                    V.tensor_tensor(t("io2"), t("iou"), t("iou"), OP.mult)
                    V.tensor_tensor(t("io3"), t("io2"), t("iou"), OP.mult)
                    V.tensor_tensor(t("junk"), t("io3"), wt[:], OP.mult)
                    V.tensor_reduce(acc4[:, ch:ch + 1], t("junk"), AX.X, OP.add)
                    if 1 <= ch <= NCHUNK - 2:
                        # buffer b is free for the next DMA round
                        V.drain().then_inc(r_sem, 1)
                    elif ch == NCHUNK - 1:
                        V.drain().then_inc(done_sem, 1)

    return nc


def _get_graph():
    if "nc" not in _cache:
        _cache["nc"] = _build_graph()
    return _cache["nc"]


def _shard_inputs(pred, target, weight):
    """Pad to NCORES*NCORE boxes and split per core."""
    per = NSHARD
    pads = NCORE - per
    pad_box = np.zeros((pads, 5), np.float32)
    pad_box[:, 2] = 1.0
    pad_box[:, 3] = 1.0
    pad_box[:, 4] = 0.3
    in_maps = []
    for c in range(NCORES):
        lo, hi = c * per, (c + 1) * per
        p = np.concatenate([np.ascontiguousarray(pred[lo:hi]), pad_box], 0)
        t = np.concatenate([np.ascontiguousarray(target[lo:hi]), pad_box], 0)
        w = np.concatenate([np.ascontiguousarray(weight[lo:hi]),
                            np.zeros(pads, np.float32)], 0)
        in_maps.append({"pred": p, "target": t, "weight": w})
    return in_maps


def kernel(pred, target, weight):
    from concourse.bass_utils import run_bass_kernel_spmd

    pred = np.asarray(pred, np.float32)
    target = np.asarray(target, np.float32)
    weight = np.asarray(weight, np.float32)

    nc = _get_graph()
    in_maps = _shard_inputs(pred, target, weight)
    res = run_bass_kernel_spmd(nc, in_maps, list(range(NCORES)))
    _cache["last_result"] = res
    total = sum(float(r["out"].astype(np.float64).sum()) for r in res.results)
    wsum = float(weight.astype(np.float64).sum())
    loss = (wsum - total) / NGLOB
    return np.float32(loss)


# revision 22
# speedup vs baseline: 1.1619x; 1.1619x over previous
"""AlphaRotatedIoULoss on 8 TRN2 NeuronCores (raw Bass SPMD kernel, v2).

Sort-free replication of the reference's rotated-IoU loss:
  - intersection area via directed-segment shoelace: clip each box's 4 edges
    against the other box (branch-free Liang-Barsky in that box's local
    frame), then sum span*cross(k,d) over the 8 directed boundary segments
    (all expressed in box2's frame).
  - the reference's shoelace drops the closing edge (last->first angle-sorted
    vertex) because invalid candidate slots are zeroed; the missing term is
    the cross of the unique boundary segment crossing the global -x ray from
    the vertex centroid.  Replicated branch-free via a global-y sign test.
Data-parallel over boxes: each core takes 1/8th, emits per-partition
per-chunk partial sums of weight*iou^3; host:  loss = (sum(w) - total) / n.

v2: F=248 (2 chunks) with manually aliased SBUF scratch, scalings folded
into scalar_tensor_tensor ops, product negations written straight into the
segment-direction slices, analytic cross terms for box2's own (axis-aligned)
edges.
"""
import numpy as np

P = 128          # partitions
T = 496          # boxes per partition per core
F = 248          # chunk width (free dim)
NCHUNK = T // F  # 2
NCORE = P * T    # 63488 boxes per core
NSHARD = 62500   # real boxes per core
NGLOB = 500000
NCORES = 8
ALPHA_EPS = 1e-6
TINY = 1e-12

_cache = {}


def _build_graph():
    import concourse.bass as bass
    import concourse.mybir as mybir
    from contextlib import ExitStack

    dt = mybir.dt.float32
    AF = mybir.ActivationFunctionType
    OP = mybir.AluOpType
    AX = mybir.AxisListType
    HALF_PI = float(np.pi / 2)

    nc = bass.Bass(detect_race_conditions=False)

    def reg_const(value):
        t = nc.alloc_sbuf_tensor(f"const-f32-{value}", [128, 1], dt)
        nc.gpsimd.memset(t.ap(), value)
        nc.const_aps.aps[(dt, float(value))] = t.ap()

    reg_const(HALF_PI)
    nc.all_engine_barrier()

    pred = nc.declare_dram_parameter("pred", [NCORE, 5], dt, isOutput=False)
    targ = nc.declare_dram_parameter("target", [NCORE, 5], dt, isOutput=False)
    wgt = nc.declare_dram_parameter("weight", [NCORE], dt, isOutput=False)
    out = nc.declare_dram_parameter("out", [P, NCHUNK], dt, isOutput=True)

    predv = pred.rearrange("(p t) f -> p t f", p=P)
    targv = targ.rearrange("(p t) f -> p t f", p=P)
    wv = wgt.rearrange("(p t) -> p t", p=P)

    V = nc.vector
    A = nc.scalar

    with ExitStack() as ctx:
        _n = [0]

        def alloc(shape):
            _n[0] += 1
            return ctx.enter_context(nc.sbuf_tensor(f"tile{_n[0]}", shape, dt))

        # double-buffered inputs (2 chunks -> one buffer each, no recycling)
        pt2 = [alloc([P, F, 5]) for _ in range(2)]
        tg2 = [alloc([P, F, 5]) for _ in range(2)]
        wt2 = [alloc([P, F]) for _ in range(2)]
        # ACT-owned trig outputs
        sdr_t, cd_t, s1_t, c1_t, s2_t, c2_t = (alloc([P, F]) for _ in range(6))
        # F-sized DVE scratch
        FN = ("delta tmpA tmpB px py W1 H1 W2 H2 ar1 ar2 ar2h sg asd sd "
              "vcf vsf gcf gsf mu mv nu nv E1f E2f F1f F2f P1f P2f Q1f Q2f "
              "m_ sx sy i2m cx cy cgy S_ CR_ inter un iou io2 io3 junk").split()
        FT = {n: alloc([P, F]) for n in FN}
        # 4F tiles
        C4 = {n: alloc([P, 4 * F]) for n in
              "KXP KXM KYP KYM RX RY GX4 GY4".split()}
        # 8F tiles
        E8 = {n: alloc([P, 8 * F]) for n in
              "K8X K8Y D8X D8Y TLO THI SPAN EA EB EC ED".split()}
        acc4 = alloc([P, NCHUNK])

        with (
            nc.semaphore("dma_sem") as dma_sem,
            nc.semaphore("v_sem") as v_sem,
            nc.semaphore("a_sem") as a_sem,
            nc.semaphore("done_sem") as done_sem,
            nc.Block() as block,
        ):
            @block.sync
            def _(sync):
                for ch in range(NCHUNK):
                    sync.dma_start(
                        out=pt2[ch][:], in_=predv[:, ch * F:(ch + 1) * F, :]
                    ).then_inc(dma_sem, 16)
                    sync.dma_start(
                        out=tg2[ch][:], in_=targv[:, ch * F:(ch + 1) * F, :]
                    ).then_inc(dma_sem, 16)
                    sync.dma_start(
                        out=wt2[ch][:], in_=wv[:, ch * F:(ch + 1) * F]
                    ).then_inc(dma_sem, 16)
                sync.wait_ge(done_sem, 1)
                sync.dma_start(out=out[:], in_=acc4[:]).then_inc(dma_sem, 16)

            @block.scalar
            def _(scalar):
                for ch in range(NCHUNK):
                    a1 = pt2[ch][:, :, 4]
                    a2 = tg2[ch][:, :, 4]
                    scalar.wait_ge(v_sem, ch + 1)
                    A.activation(sdr_t[:], FT["delta"][:], AF.Sin)
                    A.activation(cd_t[:], FT["delta"][:], AF.Sin, bias=HALF_PI)
                    A.activation(s1_t[:], a1, AF.Sin)
                    A.activation(c1_t[:], a1, AF.Sin, bias=HALF_PI)
                    A.activation(s2_t[:], a2, AF.Sin)
                    A.activation(c2_t[:], FT["tmpA"][:], AF.Sin)
                    A.drain().then_inc(a_sem, 1)

            @block.vector
            def _(vector):
                t = lambda n: FT[n][:]
                c4 = lambda n: C4[n][:]
                e8 = lambda n: E8[n][:]

                def sl(nm, i):
                    return E8[nm][:, i * F:(i + 1) * F]

                def sl4(nm, i):
                    return C4[nm][:, i * F:(i + 1) * F]

                def segreduce(dst, nm):
                    v = E8[nm][:].rearrange("p (s f) -> p f s", s=8)
                    V.tensor_reduce(dst, v, AX.X, OP.add)

                # constant zero slices of D8X/D8Y (box2's own AA edge dirs)
                V.memset(sl("D8X", 5), 0.0)
                V.memset(sl("D8X", 7), 0.0)
                V.memset(sl("D8Y", 4), 0.0)
                V.memset(sl("D8Y", 6), 0.0)

                for ch in range(NCHUNK):
                    pt, tg, wt = pt2[ch], tg2[ch], wt2[ch]
                    x1, y1, w1, h1, a1 = (pt[:, :, i] for i in range(5))
                    x2, y2, w2, h2, a2 = (tg[:, :, i] for i in range(5))

                    vector.wait_ge(dma_sem, 48 * (ch + 1))
                    # angles for ACT: delta, and wrapped a2+pi/2 in tmpA
                    V.tensor_tensor(t("delta"), a1, a2, OP.subtract)
                    V.tensor_scalar(t("tmpA"), a2, HALF_PI, None, OP.add)
                    V.tensor_scalar(t("tmpB"), t("tmpA"), float(np.pi), None,
                                    OP.is_gt)
                    V.scalar_tensor_tensor(t("tmpA"), t("tmpB"),
                                           float(-2 * np.pi), t("tmpA"),
                                           OP.mult, OP.add)
                    V.drain().then_inc(v_sem, 1)

                    # trig-independent work
                    V.tensor_tensor(t("px"), x2, x1, OP.subtract)
                    V.tensor_tensor(t("py"), y2, y1, OP.subtract)
                    V.tensor_scalar(t("W1"), w1, 0.5, None, OP.mult)
                    V.tensor_scalar(t("H1"), h1, 0.5, None, OP.mult)
                    V.tensor_scalar(t("W2"), w2, 0.5, None, OP.mult)
                    V.tensor_scalar(t("H2"), h2, 0.5, None, OP.mult)
                    V.tensor_tensor(t("ar1"), w1, h1, OP.mult)
                    V.tensor_tensor(t("ar2"), w2, h2, OP.mult)
                    V.tensor_scalar(t("ar2h"), t("ar2"), 0.5, None, OP.mult)
                    # box2 own corners -> K8X/K8Y slices 4..7
                    V.tensor_copy(sl("K8X", 4), t("W2"))
                    V.tensor_scalar(sl("K8X", 5), t("W2"), -1.0, None, OP.mult)
                    V.tensor_copy(sl("K8X", 6), sl("K8X", 5))
                    V.tensor_copy(sl("K8X", 7), t("W2"))
                    V.tensor_copy(sl("K8Y", 4), t("H2"))
                    V.tensor_copy(sl("K8Y", 5), t("H2"))
                    V.tensor_scalar(sl("K8Y", 6), t("H2"), -1.0, None, OP.mult)
                    V.tensor_copy(sl("K8Y", 7), sl("K8Y", 6))
                    # box2 own edge dirs -> D8X/D8Y slices 4..7 (x: -w2,0,w2,0)
                    V.tensor_scalar(sl("D8X", 4), w2, -1.0, None, OP.mult)
                    V.tensor_scalar(sl("D8X", 6), sl("D8X", 4), -1.0, None,
                                    OP.mult)
                    V.tensor_scalar(sl("D8Y", 5), h2, -1.0, None, OP.mult)
                    V.tensor_scalar(sl("D8Y", 7), sl("D8Y", 5), -1.0, None,
                                    OP.mult)

                    # ---- trig-dependent ------------------------------------
                    vector.wait_ge(a_sem, ch + 1)
                    cdA, s1A, c1A, s2A, c2A = (cd_t[:], s1_t[:], c1_t[:],
                                               s2_t[:], c2_t[:])
                    V.tensor_scalar(t("sg"), sdr_t[:], 0.0, None, OP.is_ge)
                    V.tensor_scalar(t("sg"), t("sg"), 2.0, -1.0, OP.mult,
                                    OP.add)
                    V.tensor_tensor(t("asd"), t("sg"), sdr_t[:], OP.mult)
                    V.tensor_scalar(t("asd"), t("asd"), TINY, None, OP.max)
                    V.tensor_tensor(t("sd"), t("sg"), t("asd"), OP.mult)
                    sdA = t("sd")

                    # box1 full products straight into D8 slices 0..3
                    wcf = sl("D8X", 2); V.tensor_tensor(wcf, w1, cdA, OP.mult)
                    V.tensor_scalar(sl("D8X", 0), wcf, -1.0, None, OP.mult)
                    wsf = sl("D8Y", 2); V.tensor_tensor(wsf, w1, sdA, OP.mult)
                    V.tensor_scalar(sl("D8Y", 0), wsf, -1.0, None, OP.mult)
                    hsf = sl("D8X", 1); V.tensor_tensor(hsf, h1, sdA, OP.mult)
                    V.tensor_scalar(sl("D8X", 3), hsf, -1.0, None, OP.mult)
                    hcf = sl("D8Y", 3); V.tensor_tensor(hcf, h1, cdA, OP.mult)
                    V.tensor_scalar(sl("D8Y", 1), hcf, -1.0, None, OP.mult)
                    # box2 full products (kept as F tiles; clip-B recips)
                    V.tensor_tensor(t("vcf"), w2, cdA, OP.mult)
                    V.tensor_tensor(t("vsf"), w2, sdA, OP.mult)
                    V.tensor_tensor(t("gcf"), h2, cdA, OP.mult)
                    V.tensor_tensor(t("gsf"), h2, sdA, OP.mult)

                    # centers
                    V.tensor_tensor(t("tmpA"), t("px"), c2A, OP.mult)
                    V.tensor_tensor(t("tmpB"), t("py"), s2A, OP.mult)
                    V.scalar_tensor_tensor(t("mu"), t("tmpA"), -1.0, t("tmpB"),
                                           OP.mult, OP.subtract)
                    V.tensor_tensor(t("tmpA"), t("px"), s2A, OP.mult)
                    V.tensor_tensor(t("tmpB"), t("py"), c2A, OP.mult)
                    V.tensor_tensor(t("mv"), t("tmpA"), t("tmpB"), OP.subtract)
                    V.tensor_tensor(t("tmpA"), t("px"), c1A, OP.mult)
                    V.tensor_tensor(t("tmpB"), t("py"), s1A, OP.mult)
                    V.tensor_tensor(t("nu"), t("tmpA"), t("tmpB"), OP.add)
                    V.tensor_tensor(t("tmpA"), t("px"), s1A, OP.mult)
                    V.tensor_tensor(t("tmpB"), t("py"), c1A, OP.mult)
                    V.tensor_tensor(t("nv"), t("tmpB"), t("tmpA"), OP.subtract)

                    # box1 corners in box2 frame (half-scales folded into STT)
                    V.tensor_tensor(t("E1f"), wcf, hsf, OP.subtract)
                    V.tensor_tensor(t("E2f"), wcf, hsf, OP.add)
                    V.tensor_tensor(t("F1f"), wsf, hcf, OP.add)
                    V.tensor_tensor(t("F2f"), wsf, hcf, OP.subtract)
                    V.scalar_tensor_tensor(sl("K8X", 0), t("E1f"), 0.5, t("mu"), OP.mult, OP.add)
                    V.scalar_tensor_tensor(sl("K8X", 1), t("E2f"), -0.5, t("mu"), OP.mult, OP.add)
                    V.scalar_tensor_tensor(sl("K8X", 2), t("E1f"), -0.5, t("mu"), OP.mult, OP.add)
                    V.scalar_tensor_tensor(sl("K8X", 3), t("E2f"), 0.5, t("mu"), OP.mult, OP.add)
                    V.scalar_tensor_tensor(sl("K8Y", 0), t("F1f"), 0.5, t("mv"), OP.mult, OP.add)
                    V.scalar_tensor_tensor(sl("K8Y", 1), t("F2f"), -0.5, t("mv"), OP.mult, OP.add)
                    V.scalar_tensor_tensor(sl("K8Y", 2), t("F1f"), -0.5, t("mv"), OP.mult, OP.add)
                    V.scalar_tensor_tensor(sl("K8Y", 3), t("F2f"), 0.5, t("mv"), OP.mult, OP.add)
                    # box2 corners in box1 frame
                    V.tensor_tensor(t("P1f"), t("vcf"), t("gsf"), OP.add)
                    V.tensor_tensor(t("P2f"), t("vcf"), t("gsf"), OP.subtract)
                    V.tensor_tensor(t("Q1f"), t("gcf"), t("vsf"), OP.subtract)
                    V.tensor_tensor(t("Q2f"), t("gcf"), t("vsf"), OP.add)
                    V.scalar_tensor_tensor(sl4("GX4", 0), t("P1f"), 0.5, t("nu"), OP.mult, OP.add)
                    V.scalar_tensor_tensor(sl4("GX4", 1), t("P2f"), -0.5, t("nu"), OP.mult, OP.add)
                    V.scalar_tensor_tensor(sl4("GX4", 2), t("P1f"), -0.5, t("nu"), OP.mult, OP.add)
                    V.scalar_tensor_tensor(sl4("GX4", 3), t("P2f"), 0.5, t("nu"), OP.mult, OP.add)
                    V.scalar_tensor_tensor(sl4("GY4", 0), t("Q1f"), 0.5, t("nv"), OP.mult, OP.add)
                    V.scalar_tensor_tensor(sl4("GY4", 1), t("Q2f"), 0.5, t("nv"), OP.mult, OP.add)
                    V.scalar_tensor_tensor(sl4("GY4", 2), t("Q1f"), -0.5, t("nv"), OP.mult, OP.add)
                    V.scalar_tensor_tensor(sl4("GY4", 3), t("Q2f"), -0.5, t("nv"), OP.mult, OP.add)

                    def emit_clip(corner_x4, corner_y4, Wb, Hb,
                                  r0, r1, r0y, r1y, rx_spec, ry_spec,
                                  lo_out, hi_out, span_out):
                        for i in range(4):
                            kx_i = corner_x4[:, i * F:(i + 1) * F]
                            ky_i = corner_y4[:, i * F:(i + 1) * F]
                            V.tensor_tensor(sl4("KXP", i), kx_i, Wb, OP.add)
                            V.tensor_tensor(sl4("KXM", i), kx_i, Wb, OP.subtract)
                            V.tensor_tensor(sl4("KYP", i), ky_i, Hb, OP.add)
                            V.tensor_tensor(sl4("KYM", i), ky_i, Hb, OP.subtract)
                        for Rt, spec, ra, rb in (("RX", rx_spec, r0, r1),
                                                 ("RY", ry_spec, r0y, r1y)):
                            (p0s, n0s), (p1s, n1s) = spec
                            V.reciprocal(sl4(Rt, p0s), ra)
                            V.reciprocal(sl4(Rt, p1s), rb)
                            V.tensor_scalar(sl4(Rt, n0s), sl4(Rt, p0s), -1.0,
                                            None, OP.mult)
                            V.tensor_scalar(sl4(Rt, n1s), sl4(Rt, p1s), -1.0,
                                            None, OP.mult)
                        # in-place: candidates overwrite KXP/KXM/KYP/KYM
                        V.tensor_tensor(c4("KXP"), c4("KXP"), c4("RX"), OP.mult)
                        V.tensor_tensor(c4("KXM"), c4("KXM"), c4("RX"), OP.mult)
                        V.tensor_tensor(c4("KYP"), c4("KYP"), c4("RY"), OP.mult)
                        V.tensor_tensor(c4("KYM"), c4("KYM"), c4("RY"), OP.mult)
                        V.tensor_tensor(c4("RX"), c4("KXP"), c4("KXM"), OP.min)
                        V.tensor_tensor(c4("KXP"), c4("KXP"), c4("KXM"), OP.max)
                        V.tensor_tensor(c4("RY"), c4("KYP"), c4("KYM"), OP.min)
                        V.tensor_tensor(c4("KYP"), c4("KYP"), c4("KYM"), OP.max)
                        V.tensor_tensor(lo_out, c4("RX"), c4("RY"), OP.max)
                        V.tensor_scalar(lo_out, lo_out, 0.0, None, OP.max)
                        V.tensor_tensor(hi_out, c4("KXP"), c4("KYP"), OP.min)
                        V.tensor_scalar(hi_out, hi_out, 1.0, None, OP.min)
                        V.scalar_tensor_tensor(span_out, lo_out, -1.0, hi_out,
                                               OP.mult, OP.add)
                        V.tensor_scalar(span_out, span_out, 0.0, None, OP.max)

                    # part A: box1 edges vs AA box2
                    emit_clip(E8["K8X"][:, 0:4 * F], E8["K8Y"][:, 0:4 * F],
                              t("W2"), t("H2"), wcf, hsf, wsf, hcf,
                              ((0, 2), (3, 1)), ((0, 2), (1, 3)),
                              E8["TLO"][:, 0:4 * F], E8["THI"][:, 0:4 * F],
                              E8["SPAN"][:, 0:4 * F])
                    # part B: box2 edges vs AA box1
                    emit_clip(c4("GX4"), c4("GY4"),
                              t("W1"), t("H1"), t("vcf"), t("gsf"),
                              t("vsf"), t("gcf"),
                              ((0, 2), (1, 3)), ((2, 0), (1, 3)),
                              E8["TLO"][:, 4 * F:8 * F],
                              E8["THI"][:, 4 * F:8 * F],
                              E8["SPAN"][:, 4 * F:8 * F])

                    # ---- area terms ----------------------------------------
                    # A half: numeric cross(k,d); B half: cross = 2*W2*H2
                    EAa = E8["EA"][:, 0:4 * F]
                    EBa = E8["EB"][:, 0:4 * F]
                    V.tensor_tensor(EAa, E8["K8X"][:, 0:4 * F],
                                    E8["D8Y"][:, 0:4 * F], OP.mult)
                    V.tensor_tensor(EBa, E8["K8Y"][:, 0:4 * F],
                                    E8["D8X"][:, 0:4 * F], OP.mult)
                    V.tensor_tensor(EAa, EAa, EBa, OP.subtract)
                    V.tensor_tensor(EBa, E8["SPAN"][:, 0:4 * F], EAa, OP.mult)
                    for i in range(4, 8):
                        V.tensor_tensor(sl("EB", i), sl("SPAN", i), t("ar2h"),
                                        OP.mult)
                    # EB = SV8 (span * cross) for all 8 segments

                    # ---- vertex centroid -----------------------------------
                    V.tensor_scalar(e8("EC"), e8("SPAN"), 0.0, None, OP.is_gt)
                    segreduce(t("m_"), "EC")
                    V.tensor_tensor(e8("ED"), e8("TLO"), e8("THI"), OP.add)
                    V.tensor_tensor(e8("THI"), e8("ED"), e8("D8X"), OP.mult)
                    V.scalar_tensor_tensor(e8("THI"), e8("K8X"), 2.0,
                                           e8("THI"), OP.mult, OP.add)
                    V.tensor_tensor(e8("THI"), e8("THI"), e8("EC"), OP.mult)
                    segreduce(t("sx"), "THI")
                    V.tensor_tensor(e8("THI"), e8("ED"), e8("D8Y"), OP.mult)
                    V.scalar_tensor_tensor(e8("THI"), e8("K8Y"), 2.0,
                                           e8("THI"), OP.mult, OP.add)
                    V.tensor_tensor(e8("THI"), e8("THI"), e8("EC"), OP.mult)
                    segreduce(t("sy"), "THI")
                    V.tensor_scalar(t("i2m"), t("m_"), 2.0, 1.0, OP.mult,
                                    OP.max)
                    V.reciprocal(t("i2m"), t("i2m"))
                    V.tensor_tensor(t("cx"), t("sx"), t("i2m"), OP.mult)
                    V.tensor_tensor(t("cy"), t("sy"), t("i2m"), OP.mult)

                    # ---- global-y of starts/dirs (KGY->EC, DGY->ED) --------
                    V.tensor_tensor(sl("ED", 2), w1, s1A, OP.mult)   # wsg
                    V.tensor_scalar(sl("ED", 0), sl("ED", 2), -1.0, None, OP.mult)
                    V.tensor_tensor(sl("ED", 3), h1, c1A, OP.mult)   # hcg
                    V.tensor_scalar(sl("ED", 1), sl("ED", 3), -1.0, None, OP.mult)
                    V.tensor_tensor(sl("ED", 6), w2, s2A, OP.mult)   # vsg
                    V.tensor_scalar(sl("ED", 4), sl("ED", 6), -1.0, None, OP.mult)
                    V.tensor_tensor(sl("ED", 7), h2, c2A, OP.mult)   # gcg
                    V.tensor_scalar(sl("ED", 5), sl("ED", 7), -1.0, None, OP.mult)
                    # S combos reuse E1f..F2f
                    V.tensor_tensor(t("E1f"), sl("ED", 2), sl("ED", 3), OP.add)
                    V.tensor_tensor(t("E2f"), sl("ED", 3), sl("ED", 2), OP.subtract)
                    V.tensor_tensor(t("F1f"), sl("ED", 6), sl("ED", 7), OP.add)
                    V.tensor_tensor(t("F2f"), sl("ED", 7), sl("ED", 6), OP.subtract)
                    V.scalar_tensor_tensor(sl("EC", 0), t("E1f"), 0.5, t("py"), OP.mult, OP.subtract)
                    V.scalar_tensor_tensor(sl("EC", 1), t("E2f"), 0.5, t("py"), OP.mult, OP.subtract)
                    V.scalar_tensor_tensor(sl("EC", 2), t("E1f"), -0.5, t("py"), OP.mult, OP.subtract)
                    V.scalar_tensor_tensor(sl("EC", 3), t("E2f"), -0.5, t("py"), OP.mult, OP.subtract)
                    V.tensor_scalar(sl("EC", 4), t("F1f"), 0.5, None, OP.mult)
                    V.tensor_scalar(sl("EC", 5), t("F2f"), 0.5, None, OP.mult)
                    V.tensor_scalar(sl("EC", 6), t("F1f"), -0.5, None, OP.mult)
                    V.tensor_scalar(sl("EC", 7), t("F2f"), -0.5, None, OP.mult)
                    # centroid global-y
                    V.tensor_tensor(t("tmpA"), s2A, t("cx"), OP.mult)
                    V.tensor_tensor(t("tmpB"), c2A, t("cy"), OP.mult)
                    V.tensor_tensor(t("cgy"), t("tmpA"), t("tmpB"), OP.add)

                    # ---- tau tests + correction ----------------------------
                    # TAU_A -> THI, KGYR -> EA (per slice), TAU_B -> TLO
                    V.tensor_tensor(e8("THI"), e8("TLO"), e8("ED"), OP.mult)
                    for i in range(8):
                        V.tensor_tensor(sl("EA", i), sl("EC", i), t("cgy"),
                                        OP.subtract)
                    V.tensor_tensor(e8("THI"), e8("THI"), e8("EA"), OP.add)
                    V.tensor_tensor(e8("TLO"), e8("SPAN"), e8("ED"), OP.mult)
                    V.tensor_tensor(e8("TLO"), e8("TLO"), e8("THI"), OP.add)
                    V.tensor_scalar(e8("EC"), e8("THI"), 0.0, None, OP.is_ge)
                    V.tensor_scalar(e8("THI"), e8("TLO"), 0.0, None, OP.is_lt)
                    V.tensor_tensor(e8("EC"), e8("EC"), e8("THI"), OP.mult)
                    # chi = cross(c, d): EA = cx*D8Y, THI = cy*D8X (per slice)
                    for i in range(8):
                        V.tensor_tensor(sl("EA", i), t("cx"), sl("D8Y", i),
                                        OP.mult)
                        V.tensor_tensor(sl("THI", i), t("cy"), sl("D8X", i),
                                        OP.mult)
                    V.tensor_tensor(e8("EA"), e8("EA"), e8("THI"), OP.subtract)
                    V.tensor_tensor(e8("EA"), e8("SPAN"), e8("EA"), OP.mult)
                    V.tensor_tensor(e8("EA"), e8("EB"), e8("EA"), OP.subtract)
                    V.tensor_tensor(e8("EA"), e8("EC"), e8("EA"), OP.mult)

                    segreduce(t("S_"), "EB")
                    segreduce(t("CR_"), "EA")
                    V.tensor_tensor(t("S_"), t("S_"), t("CR_"), OP.subtract)

                    # ---- iou / loss ----------------------------------------
                    V.tensor_scalar(t("inter"), t("S_"), 0.5, 0.0, OP.mult,
                                    OP.max)
                    V.tensor_tensor(t("un"), t("ar1"), t("ar2"), OP.add)
                    V.tensor_tensor(t("un"), t("un"), t("inter"), OP.subtract)
                    V.tensor_scalar(t("un"), t("un"), ALPHA_EPS, None, OP.max)
                    V.reciprocal(t("un"), t("un"))
                    V.tensor_tensor(t("iou"), t("inter"), t("un"), OP.mult)
                    V.tensor_scalar(t("iou"), t("iou"), ALPHA_EPS, None, OP.max)
                    V.tensor_tensor(t("io2"), t("iou"), t("iou"), OP.mult)
                    V.tensor_tensor(t("io3"), t("io2"), t("iou"), OP.mult)
                    V.tensor_tensor(t("junk"), t("io3"), wt[:], OP.mult)
                    V.tensor_reduce(acc4[:, ch:ch + 1], t("junk"), AX.X, OP.add)
                    if ch == NCHUNK - 1:
                        V.drain().then_inc(done_sem, 1)

    return nc


def _get_graph():
    if "nc" not in _cache:
        _cache["nc"] = _build_graph()
    return _cache["nc"]


def _shard_inputs(pred, target, weight):
    """Pad to NCORES*NCORE boxes and split per core."""
    per = NSHARD
    pads = NCORE - per
    pad_box = np.zeros((pads, 5), np.float32)
    pad_box[:, 2] = 1.0
    pad_box[:, 3] = 1.0
    pad_box[:, 4] = 0.3
    in_maps = []
    for c in range(NCORES):
        lo, hi = c * per, (c + 1) * per
        p = np.concatenate([np.ascontiguousarray(pred[lo:hi]), pad_box], 0)
        t = np.concatenate([np.ascontiguousarray(target[lo:hi]), pad_box], 0)
        w = np.concatenate([np.ascontiguousarray(weight[lo:hi]),
                            np.zeros(pads, np.float32)], 0)
        in_maps.append({"pred": p, "target": t, "weight": w})
    return in_maps


def kernel(pred, target, weight):
    from concourse.bass_utils import run_bass_kernel_spmd

    pred = np.asarray(pred, np.float32)
    target = np.asarray(target, np.float32)
    weight = np.asarray(weight, np.float32)

    nc = _get_graph()
    in_maps = _shard_inputs(pred, target, weight)
    res = run_bass_kernel_spmd(nc, in_maps, list(range(NCORES)))
    _cache["last_result"] = res
    total = sum(float(r["out"].astype(np.float64).sum()) for r in res.results)
    wsum = float(weight.astype(np.float64).sum())
    loss = (wsum - total) / NGLOB
    return np.float32(loss)
